# revision 5
# baseline (speedup 1.0000x reference)
"""Self-contained Trainium2 Bass kernel for nn_MiMoMoeAttention.

Tensor-parallel over heads across 8 NeuronCores: each core owns 4 query
heads + 1 kv head (one GQA group); q/k/v projections column-sharded,
o_proj row-sharded, f16 partial outputs summed on the host.

All matmuls f16 (1 col/cycle on the PE). Structured to keep the PE
p-state at max clock: no gaps between qkv -> o_proj(prev) -> attention.
 - hidden states prefetched a full chunk ahead in half-chunk DMAs
 - Wo resident in SBUF (loaded once); startup DMAs ordered so the first
   matmul fires after ~1.5MB of transfers
 - RoPE fused on the vector engine: (psum + bias) * cos via
   scalar_tensor_tensor, half-swap via cross-partition copies
 - RoPE for the first head pair emitted before o_proj(prev) so the
   attention pass starts the moment o_proj's matmuls drain
 - o_proj PSUM->SBUF copies all on the scalar engine (Copy shares the
   Exp activation table - no table reloads)
 - V transposed by the DMA engine (dma_start_transpose), not the PE
 - softmax denominators for a head pair accumulate into one PSUM bank
   (partitions 0/64); reciprocal via vector custom op
"""
import numpy as np
from contextlib import ExitStack


from concourse import bacc
import concourse.tile as tile
import concourse.mybir as mybir
from concourse.alu_op_type import AluOpType
from concourse.bass_utils import run_bass_kernel_spmd

dt = mybir.dt
AF = mybir.ActivationFunctionType

B, S, HID = 1, 2048, 4096
H, HK, D = 32, 8, 128
WIN = 1024
THETA = 1000000.0
NCORES = 8
HQ = H // NCORES            # 4 query heads per core
CH = 512                    # token chunk width
NCH = S // CH               # 4 chunks
KT = HID // 128             # 32 contraction tiles
KH = KT // 2                # 16 tiles per half-chunk DMA
NE = HID // CH              # 8 o_proj column chunks
CBLK = CH // 128            # 4 query blocks per chunk
WBLK = WIN // 128           # 8 blocks lookback
SCALE = float(D) ** -0.5


def _build():
    nc = bacc.Bacc("TRN2", target_bir_lowering=False, debug=False,
                   num_devices=NCORES)
    f32, f16 = dt.float32, dt.float16
    # hsT half-chunk tiled: row block (2c+half) holds [128, KH*CH] with
    # kt-within-half major columns
    hsT = nc.dram_tensor("hsT", [NCH * 2 * 128, KH * CH], f16,
                         kind="ExternalInput").ap()
    wq = nc.dram_tensor("wq", [128, KT * HQ * D], f16,
                        kind="ExternalInput").ap()
    wk = nc.dram_tensor("wk", [128, KT * D], f16, kind="ExternalInput").ap()
    wv = nc.dram_tensor("wv", [128, KT * D], f16, kind="ExternalInput").ap()
    # wo flat: [128, (jt*NE + e)*CH + x] = Wo[jt*128 + p, e*CH + x]
    wo = nc.dram_tensor("wo", [128, HQ * NE * CH], f16,
                        kind="ExternalInput").ap()
    bq = nc.dram_tensor("bq", [128, HQ], f32, kind="ExternalInput").ap()
    bqs = nc.dram_tensor("bqs", [128, HQ], f32, kind="ExternalInput").ap()
    bk = nc.dram_tensor("bk", [128, 1], f32, kind="ExternalInput").ap()
    bks = nc.dram_tensor("bks", [128, 1], f32, kind="ExternalInput").ap()
    bv = nc.dram_tensor("bv", [128, 1], f32, kind="ExternalInput").ap()
    cosd = nc.dram_tensor("cosd", [128, S], f16, kind="ExternalInput").ap()
    sind = nc.dram_tensor("sind", [128, S], f16, kind="ExternalInput").ap()
    m0 = nc.dram_tensor("m0", [128, 128], f16, kind="ExternalInput").ap()
    m8 = nc.dram_tensor("m8", [128, 128], f16, kind="ExternalInput").ap()
    # esink[:, pass]: exp(sink) of the pass's even head at partition 0 and
    # odd head at partition 64; 1.0 elsewhere
    esink = nc.dram_tensor("esink", [128, 2], f32, kind="ExternalInput").ap()
    onc = nc.dram_tensor("onc", [128, 1], f16, kind="ExternalInput").ap()
    onr = nc.dram_tensor("onr", [128, 128], f16, kind="ExternalInput").ap()
    out = nc.dram_tensor("o_part", [S, HID], f16, kind="ExternalOutput").ap()

    with tile.TileContext(nc) as tc, ExitStack() as ctx:
        const = ctx.enter_context(tc.tile_pool(name="const", bufs=1))
        keep = ctx.enter_context(tc.tile_pool(name="keep", bufs=1))
        work = ctx.enter_context(tc.tile_pool(name="work", bufs=1))
        ps = ctx.enter_context(tc.tile_pool(name="ps", bufs=1, space="PSUM"))

        # ---- tiny constants first --------------------------------------
        bq_sb = const.tile([128, HQ], f32, tag="bq", name="bq_sb")
        nc.sync.dma_start(bq_sb[:], bq)
        bqs_sb = const.tile([128, HQ], f32, tag="bqs", name="bqs_sb")
        nc.sync.dma_start(bqs_sb[:], bqs)
        bk_sb = const.tile([128, 1], f32, tag="bk", name="bk_sb")
        nc.sync.dma_start(bk_sb[:], bk)
        bks_sb = const.tile([128, 1], f32, tag="bks", name="bks_sb")
        nc.sync.dma_start(bks_sb[:], bks)
        bv_sb = const.tile([128, 1], f32, tag="bv", name="bv_sb")
        nc.sync.dma_start(bv_sb[:], bv)
        m0_sb = const.tile([128, 128], f16, tag="m0", name="m0_sb")
        nc.sync.dma_start(m0_sb[:], m0)
        m8_sb = const.tile([128, 128], f16, tag="m8", name="m8_sb")
        nc.sync.dma_start(m8_sb[:], m8)
        es_sb = const.tile([128, 2], f32, tag="es", name="es_sb")
        nc.sync.dma_start(es_sb[:], esink)
        ones_c = const.tile([128, 1], f16, tag="ones_c", name="ones_c")
        nc.sync.dma_start(ones_c[:], onc)
        ones_r = const.tile([128, 128], f16, tag="ones_r", name="ones_r")
        nc.sync.dma_start(ones_r[:], onr)

        # ---- weights/hidden: just enough for the first matmuls ---------
        wq_sb = const.tile([128, KT * HQ * D], f16, tag="wq", name="wq_sb")
        wk_sb = const.tile([128, KT * D], f16, tag="wk", name="wk_sb")
        wv_sb = const.tile([128, KT * D], f16, tag="wv", name="wv_sb")
        QW = KT * HQ * D // 8
        KW = KT * D // 2
        nc.sync.dma_start(wq_sb[:, 0:QW], wq[:, 0:QW])
        nc.sync.dma_start(wk_sb[:, 0:KW], wk[:, 0:KW])
        nc.sync.dma_start(wv_sb[:, 0:KW], wv[:, 0:KW])

        hst_t = {}

        def fetch_half(h, split=1):
            t = work.tile([128, KH * CH], f16, tag="hst", bufs=3,
                          name=f"hst{h}")
            w = KH * CH // split
            for i in range(split):
                nc.sync.dma_start(t[:, i * w:(i + 1) * w],
                                  hsT[h * 128:(h + 1) * 128, i * w:(i + 1) * w])
            hst_t[h] = t

        fetch_half(0, split=8)
        nc.sync.dma_start(wq_sb[:, QW:2 * QW], wq[:, QW:2 * QW])
        cos_sb = const.tile([128, S], f16, tag="cos", name="cos_sb")
        nc.sync.dma_start(cos_sb[:], cosd)
        sin_sb = const.tile([128, S], f16, tag="sin", name="sin_sb")
        nc.sync.dma_start(sin_sb[:], sind)
        nc.sync.dma_start(wq_sb[:, 2 * QW:4 * QW], wq[:, 2 * QW:4 * QW])
        nc.sync.dma_start(wk_sb[:, KW:], wk[:, KW:])
        nc.sync.dma_start(wv_sb[:, KW:], wv[:, KW:])
        nc.sync.dma_start(wq_sb[:, 4 * QW:], wq[:, 4 * QW:])
        fetch_half(1)

        # ---- wo resident (needed first at chunk 1) ---------------------
        wo_sb = const.tile([128, HQ * NE * CH], f16, tag="wo", name="wo_sb")
        nc.sync.dma_start(wo_sb[:], wo)

        # persistent rotated K (d-major) and V (t-major) for all tokens
        krotT = keep.tile([128, S], f16, tag="krotT", name="krotT")
        v_all = keep.tile([128, S], f16, tag="v_all", name="v_all")

        def rope(dst, src_ps, b_col, bs_col, s0):
            """dst = (src+b)*cos + swap(src+b)*sinS, straight from PSUM.
            sinS rows 0:64 = -sin, rows 64:128 = +sin; swap(b) passed in."""
            t1 = work.tile([128, CH], f16, tag="r1", bufs=2, name="t1")
            nc.vector.scalar_tensor_tensor(
                t1[:], src_ps[:], b_col, cos_sb[:, s0:s0 + CH],
                op0=AluOpType.add, op1=AluOpType.mult)
            sw = work.tile([128, CH], f16, tag="rsw", bufs=2, name="sw")
            nc.vector.tensor_copy(sw[0:64, :], src_ps[64:128, :])
            nc.vector.tensor_copy(sw[64:128, :], src_ps[0:64, :])
            t2 = work.tile([128, CH], f16, tag="r2", bufs=2, name="t2")
            nc.vector.scalar_tensor_tensor(
                t2[:], sw[:], bs_col, sin_sb[:, s0:s0 + CH],
                op0=AluOpType.add, op1=AluOpType.mult)
            nc.vector.tensor_tensor(dst, t1[:], t2[:], op=AluOpType.add)

        def attn_pass(pi, heads, qrot, blo, bhi, an_out):
            """Sliding-window attention for a head pair, j-outer,
            software-pipelined. Denominators for both heads accumulate in
            one PSUM bank at partitions 0 / 64."""
            lg_tags = ("c0", "c1", "a3") if pi == 0 else \
                      ("c0", "c1", "a3", "b0", "b1")
            at = {}
            for idx, h in enumerate(heads):
                at[h] = ps.tile([128, CH], f32, tag=f"a{idx}", name="at_ps")
                nc.vector.memset(at[h][:], 0.0)
            dn = ps.tile([128, CH], f32, tag="a2", name="dn_ps")
            nc.vector.memset(dn[:], 0.0)
            lgi = 0
            pend = []

            def consume(item):
                idx, h, E, c0, w, j = item
                p = 64 * idx
                nc.tensor.matmul(dn[p:p + 1, c0:c0 + w], ones_c[:], E[:, :w],
                                 start=False, stop=True)
                nc.tensor.matmul(at[h][:, c0:c0 + w],
                                 v_all[:, j * 128:(j + 1) * 128],
                                 E[:, :w], start=False, stop=True)

            for j in range(max(0, blo - WBLK), bhi + 1):
                lo, hi = max(j, blo), min(j + WBLK, bhi)
                c0 = (lo - blo) * 128
                w = (hi - lo + 1) * 128
                has_m0 = lo == j
                has_m8 = hi == j + WBLK
                for idx, h in enumerate(heads):
                    lg = ps.tile([128, CH], f32, tag=lg_tags[lgi % len(lg_tags)],
                                 name="lg")
                    lgi += 1
                    nc.tensor.matmul(lg[:, :w],
                                     krotT[:, j * 128:(j + 1) * 128],
                                     qrot[h][:, c0:c0 + w],
                                     start=True, stop=True)
                    E = work.tile([128, CH], f16, tag="E", bufs=10, name="E")
                    nc.scalar.activation(E[:, :w], lg[:, :w], AF.Exp,
                                         scale=SCALE)
                    if has_m0:
                        nc.vector.tensor_tensor(E[:, 0:128], E[:, 0:128],
                                                m0_sb[:], op=AluOpType.mult)
                    if has_m8:
                        nc.vector.tensor_tensor(E[:, w - 128:w],
                                                E[:, w - 128:w],
                                                m8_sb[:], op=AluOpType.mult)
                    pend.append((idx, h, E, c0, w, j))
                while len(pend) > len(lg_tags) + 1:
                    consume(pend.pop(0))
            for item in pend:
                consume(item)

            # normalization: rcp = 1/(dn + esink) batched for both heads
            dnb = work.tile([128, CH], f32, tag="dnb", bufs=2, name="dnb")
            nc.vector.tensor_scalar_add(dnb[:], dn[:], es_sb[:, pi:pi + 1])
            rcp = work.tile([128, CH], f32, tag="rcp", bufs=2, name="rcp")
            nc.vector.reciprocal_approx_fast(rcp[:], dnb[:])
            rcp16 = work.tile([128, CH], f16, tag="rcp16", bufs=2,
                              name="rcp16")
            nc.vector.tensor_copy(rcp16[:], rcp[:])
            for idx, h in enumerate(heads):
                p = 64 * idx
                rb_ps = ps.tile([128, CH], f32, tag=("c0", "c1")[idx],
                                name="rb_ps")
                nc.tensor.matmul(rb_ps[:], ones_r[p:p + 1, :],
                                 rcp16[p:p + 1, :], start=True, stop=True)
                rb_sb = work.tile([128, CH], f32, tag="rb", bufs=2,
                                  name="rb_sb")
                nc.vector.tensor_copy(rb_sb[:], rb_ps[:])
                an = work.tile([128, CH], f16, tag="an", bufs=8, name="an")
                nc.vector.tensor_tensor(an[:], at[h][:], rb_sb[:],
                                        op=AluOpType.mult)
                an_out[h] = an

        def o_proj(s0, an_out):
            for e in range(NE):
                for sb in range(CBLK):
                    o_ps = ps.tile([128, CH], f32,
                                   tag=("c0", "c1")[(e * CBLK + sb) % 2],
                                   name="o_ps")
                    for jt in range(HQ):
                        nc.tensor.matmul(
                            o_ps[:], an_out[jt][:, sb * 128:(sb + 1) * 128],
                            wo_sb[:, (jt * NE + e) * CH:(jt * NE + e + 1) * CH],
                            start=jt == 0, stop=jt == HQ - 1)
                    o_sb = work.tile([128, CH], f16, tag="osb", bufs=4,
                                     name="o_sb")
                    nc.scalar.copy(o_sb[:], o_ps[:])
                    nc.sync.dma_start(
                        out[s0 + sb * 128:s0 + (sb + 1) * 128,
                            e * CH:(e + 1) * CH], o_sb[:])

        QTAG = ("a0", "a1", "b0", "b1")
        prev = None
        for c in range(NCH):
            s0 = c * CH

            # ---- fused q/k/v projection for this token chunk ----------
            q_ps = [ps.tile([128, CH], f32, tag=QTAG[jt], name=f"q_ps{jt}")
                    for jt in range(HQ)]
            k_ps = ps.tile([128, CH], f32, tag="a2", name="k_ps")
            v_ps = ps.tile([128, CH], f32, tag="a3", name="v_ps")
            for kt in range(KT):
                if kt == 0 and c + 1 < NCH:
                    fetch_half(2 * (c + 1))
                if kt == KH and c + 1 < NCH:
                    fetch_half(2 * (c + 1) + 1)
                hs_kt = hst_t[2 * c + kt // KH][
                    :, (kt % KH) * CH:(kt % KH + 1) * CH]
                first, last = kt == 0, kt == KT - 1
                for jt in range(HQ):
                    nc.tensor.matmul(
                        q_ps[jt][:],
                        wq_sb[:, kt * HQ * D + jt * D:kt * HQ * D + (jt + 1) * D],
                        hs_kt, start=first, stop=last)
                nc.tensor.matmul(k_ps[:], wk_sb[:, kt * D:(kt + 1) * D],
                                 hs_kt, start=first, stop=last)
                nc.tensor.matmul(v_ps[:], wv_sb[:, kt * D:(kt + 1) * D],
                                 hs_kt, start=first, stop=last)

            # ---- K + first head pair rope (vector), V bias+DMA-transpose
            # ---- queued before o_proj so attention starts right after it
            rope(krotT[:, s0:s0 + CH], k_ps, bk_sb[:], bks_sb[:], s0)
            qrot = {}
            for jt in (0, 1):
                qr = work.tile([128, CH], f16, tag="qrot", bufs=4, name="qr")
                rope(qr[:], q_ps[jt], bq_sb[:, jt:jt + 1],
                     bqs_sb[:, jt:jt + 1], s0)
                qrot[jt] = qr
            vT_sb = work.tile([128, CH], f16, tag="vt", bufs=2, name="vT_sb")
            nc.vector.tensor_scalar_add(vT_sb[:], v_ps[:], bv_sb[:])
            for i in range(CBLK):
                nc.sync.dma_start_transpose(
                    v_all[:, s0 + i * 128:s0 + (i + 1) * 128],
                    vT_sb[:, i * 128:(i + 1) * 128])

            # ---- o_proj of PREVIOUS chunk keeps the PE busy while the
            # ---- vector engine ropes ----------------------------------
            if prev is not None:
                o_proj(prev[0], prev[1])

            # ---- attention passes; second head pair ropes in between --
            blo, bhi = c * CBLK, c * CBLK + CBLK - 1
            an_out = {}
            attn_pass(0, (0, 1), qrot, blo, bhi, an_out)
            for jt in (2, 3):
                qr = work.tile([128, CH], f16, tag="qrot", bufs=4, name="qr")
                rope(qr[:], q_ps[jt], bq_sb[:, jt:jt + 1],
                     bqs_sb[:, jt:jt + 1], s0)
                qrot[jt] = qr
            attn_pass(1, (2, 3), qrot, blo, bhi, an_out)
            prev = (s0, an_out)
        o_proj(prev[0], prev[1])

    nc.compile()
    return nc


_CACHED = None
_LAST_IN_MAPS = None


def _get_nc():
    global _CACHED
    if _CACHED is None:
        _CACHED = _build()
    return _CACHED


def kernel(positions, hidden_states, Wq, bq, Wk, bk, Wv, bv, Wo, sink,
           **_ignored):
    positions = np.asarray(positions)
    hidden_states = np.asarray(hidden_states, dtype=np.float32)
    Wq = np.asarray(Wq, dtype=np.float32)
    Wk = np.asarray(Wk, dtype=np.float32)
    Wv = np.asarray(Wv, dtype=np.float32)
    Wo = np.asarray(Wo, dtype=np.float32)
    bq = np.asarray(bq, dtype=np.float32)
    bk = np.asarray(bk, dtype=np.float32)
    bv = np.asarray(bv, dtype=np.float32)
    sink = np.asarray(sink, dtype=np.float32)

    # host-derived tables
    half = D // 2
    inv_freq = 1.0 / (THETA ** (np.arange(half, dtype=np.float64) / half))
    ang = positions[0].astype(np.float64)[None, :] * inv_freq[:, None]  # [64,S]
    cos64 = np.cos(ang).astype(np.float16)
    sin64 = np.sin(ang)
    cosd = np.ascontiguousarray(np.concatenate([cos64, cos64], axis=0))
    # signed sin: top half -sin (x1c - x2s), bottom half +sin (x2c + x1s)
    sind = np.ascontiguousarray(
        np.concatenate([-sin64, sin64], axis=0)).astype(np.float16)
    r, cidx = np.arange(128)[:, None], np.arange(128)[None, :]
    m0 = (r <= cidx).astype(np.float16)
    m8 = (r > cidx).astype(np.float16)

    def swap_halves(b):  # [D] -> halves exchanged
        return np.concatenate([b[half:], b[:half]])

    # hsT half-chunk tiled: [NCH*2*128, KH*CH]
    hsT_full = np.ascontiguousarray(hidden_states[0].T).astype(np.float16)
    hsT_t = np.ascontiguousarray(
        hsT_full.reshape(2, KH, 128, NCH, CH).transpose(3, 0, 2, 1, 4)
        .reshape(NCH * 2 * 128, KH * CH))
    esink_all = np.exp(sink.astype(np.float64)).astype(np.float32)

    in_maps = []
    for core in range(NCORES):
        qs = slice(core * HQ * D, (core + 1) * HQ * D)
        ks = slice(core * D, (core + 1) * D)
        # weights pre-tiled: [128, KT*cols] with kt blocks side by side
        wq_c = Wq[:, qs].astype(np.float16).reshape(KT, 128, HQ * D)
        wq_t = np.ascontiguousarray(
            wq_c.transpose(1, 0, 2).reshape(128, KT * HQ * D))
        wk_t = np.ascontiguousarray(
            Wk[:, ks].astype(np.float16).reshape(KT, 128, D)
            .transpose(1, 0, 2).reshape(128, KT * D))
        wv_t = np.ascontiguousarray(
            Wv[:, ks].astype(np.float16).reshape(KT, 128, D)
            .transpose(1, 0, 2).reshape(128, KT * D))
        # wo flat: [128, (jt*NE + e)*CH + x]
        wo_t = np.ascontiguousarray(
            Wo[qs, :].astype(np.float16).reshape(HQ, 128, NE * CH)
            .transpose(1, 0, 2).reshape(128, HQ * NE * CH))
        bq_c = bq[qs].reshape(HQ, D)
        bqs_c = np.stack([swap_halves(b) for b in bq_c])
        es = np.ones((128, 2), dtype=np.float32)
        for pi in range(2):
            es[0, pi] = esink_all[core * HQ + 2 * pi]
            es[64, pi] = esink_all[core * HQ + 2 * pi + 1]
        in_maps.append(dict(
            hsT=hsT_t, wq=wq_t, wk=wk_t, wv=wv_t, wo=wo_t,
            bq=np.ascontiguousarray(bq_c.T),
            bqs=np.ascontiguousarray(bqs_c.T),
            bk=np.ascontiguousarray(bk[ks].reshape(D, 1)),
            bks=np.ascontiguousarray(swap_halves(bk[ks]).reshape(D, 1)),
            bv=np.ascontiguousarray(bv[ks].reshape(D, 1)),
            cosd=cosd, sind=sind, m0=m0, m8=m8,
            esink=es,
            onc=np.ones((128, 1), dtype=np.float16),
            onr=np.ones((128, 128), dtype=np.float16),
        ))

    global _LAST_IN_MAPS
    _LAST_IN_MAPS = in_maps
    nc = _get_nc()
    res = None
    for attempt in range(3):
        try:
            res = run_bass_kernel_spmd(nc, in_maps, list(range(NCORES)))
            break
        except Exception:
            if attempt == 2:
                raise
            import time as _t
            _t.sleep(2.0)
    out = np.zeros((S, HID), dtype=np.float64)
    for core in range(NCORES):
        out += res.results[core]["o_part"].astype(np.float64)
    return out.astype(np.float32).reshape(B, S, HID)


# revision 6
# speedup vs baseline: 1.0030x; 1.0030x over previous
"""Self-contained Trainium2 Bass kernel for nn_MiMoMoeAttention.

Tensor-parallel over heads across 8 NeuronCores: each core owns 4 query
heads + 1 kv head (one GQA group); q/k/v projections column-sharded,
o_proj row-sharded, f16 partial outputs summed on the host.

All matmuls f16 (1 col/cycle on the PE, ~2.4GHz when streaming without
gaps). Structured so the PE never idles at phase boundaries:
 - hidden states prefetched a full chunk ahead in half-chunk DMAs;
   startup DMAs ordered so the first matmul fires after ~1.5MB
 - Wo resident in SBUF (loaded once)
 - RoPE fused on the vector engine: (psum + bias) * cos via
   scalar_tensor_tensor, half-swap via cross-partition copies
 - o_proj of chunk c-1 emitted right after chunk c's qkv matmuls so the
   PE stays busy while the vector engine ropes
 - attention accumulators copied to SBUF at the pend flush so all six
   qkv PSUM banks are free the moment a pass's matmuls drain; the
   softmax normalization chain then overlaps the next chunk's qkv
 - softmax denominators for a head pair accumulate into one PSUM bank
   (partitions 0/64); reciprocal via vector custom op
"""
import numpy as np
from contextlib import ExitStack


from concourse import bacc
import concourse.tile as tile
import concourse.mybir as mybir
from concourse.alu_op_type import AluOpType
from concourse.bass_utils import run_bass_kernel_spmd

dt = mybir.dt
AF = mybir.ActivationFunctionType

B, S, HID = 1, 2048, 4096
H, HK, D = 32, 8, 128
WIN = 1024
THETA = 1000000.0
NCORES = 8
HQ = H // NCORES            # 4 query heads per core
CH = 512                    # token chunk width
NCH = S // CH               # 4 chunks
KT = HID // 128             # 32 contraction tiles
KH = KT // 2                # 16 tiles per half-chunk DMA
NE = HID // CH              # 8 o_proj column chunks
CBLK = CH // 128            # 4 query blocks per chunk
WBLK = WIN // 128           # 8 blocks lookback
SCALE = float(D) ** -0.5


def _build():
    nc = bacc.Bacc("TRN2", target_bir_lowering=False, debug=False,
                   num_devices=NCORES)
    f32, f16 = dt.float32, dt.float16
    # hsT half-chunk tiled: row block (2c+half) holds [128, KH*CH] with
    # kt-within-half major columns
    hsT = nc.dram_tensor("hsT", [NCH * 2 * 128, KH * CH], f16,
                         kind="ExternalInput").ap()
    wq = nc.dram_tensor("wq", [128, KT * HQ * D], f16,
                        kind="ExternalInput").ap()
    wk = nc.dram_tensor("wk", [128, KT * D], f16, kind="ExternalInput").ap()
    wv = nc.dram_tensor("wv", [128, KT * D], f16, kind="ExternalInput").ap()
    # wo flat: [128, (jt*NE + e)*CH + x] = Wo[jt*128 + p, e*CH + x]
    wo = nc.dram_tensor("wo", [128, HQ * NE * CH], f16,
                        kind="ExternalInput").ap()
    # packed constants: f32 [bq(4) bqs(4) bk bks bv es(2)] = 13 cols;
    # f16 [m0(128) m8(128) onc(1) onr(128) ident(128)] = 513 cols
    cf32 = nc.dram_tensor("cf32", [128, 13], f32, kind="ExternalInput").ap()
    cf16 = nc.dram_tensor("cf16", [128, 513], f16, kind="ExternalInput").ap()
    cosd = nc.dram_tensor("cosd", [128, S], f16, kind="ExternalInput").ap()
    sind = nc.dram_tensor("sind", [128, S], f16, kind="ExternalInput").ap()
    out = nc.dram_tensor("o_part", [S, HID], f16, kind="ExternalOutput").ap()

    with tile.TileContext(nc) as tc, ExitStack() as ctx:
        const = ctx.enter_context(tc.tile_pool(name="const", bufs=1))
        keep = ctx.enter_context(tc.tile_pool(name="keep", bufs=1))
        work = ctx.enter_context(tc.tile_pool(name="work", bufs=1))
        ps = ctx.enter_context(tc.tile_pool(name="ps", bufs=1, space="PSUM"))

        # ---- packed constants (2 DMAs) ---------------------------------
        c32_sb = const.tile([128, 13], f32, tag="c32", name="c32_sb")
        nc.sync.dma_start(c32_sb[:], cf32)
        c16_sb = const.tile([128, 513], f16, tag="c16", name="c16_sb")
        nc.sync.dma_start(c16_sb[:], cf16)
        bq_sb = c32_sb[:, 0:4]
        bqs_sb = c32_sb[:, 4:8]
        bk_sb = c32_sb[:, 8:9]
        bks_sb = c32_sb[:, 9:10]
        bv_sb = c32_sb[:, 10:11]
        es_sb = c32_sb[:, 11:13]
        m0_sb = c16_sb[:, 0:128]
        m8_sb = c16_sb[:, 128:256]
        ones_c = c16_sb[:, 256:257]
        ones_r = c16_sb[:, 257:385]
        idf_sb = c16_sb[:, 385:513]

        # ---- hidden/weights: just enough for the first matmuls ---------
        wq_sb = const.tile([128, KT * HQ * D], f16, tag="wq", name="wq_sb")
        wk_sb = const.tile([128, KT * D], f16, tag="wk", name="wk_sb")
        wv_sb = const.tile([128, KT * D], f16, tag="wv", name="wv_sb")
        QW = KT * HQ * D // 8
        KW = KT * D // 2

        hst_t = {}

        def fetch_half(h, split=1):
            t = work.tile([128, KH * CH], f16, tag="hst", bufs=3,
                          name=f"hst{h}")
            w = KH * CH // split
            for i in range(split):
                nc.sync.dma_start(t[:, i * w:(i + 1) * w],
                                  hsT[h * 128:(h + 1) * 128, i * w:(i + 1) * w])
            hst_t[h] = t

        fetch_half(0, split=8)
        nc.sync.dma_start(wq_sb[:, 0:QW], wq[:, 0:QW])
        nc.sync.dma_start(wk_sb[:, 0:KW], wk[:, 0:KW])
        nc.sync.dma_start(wv_sb[:, 0:KW], wv[:, 0:KW])
        nc.sync.dma_start(wq_sb[:, QW:2 * QW], wq[:, QW:2 * QW])
        cos_sb = const.tile([128, S], f16, tag="cos", name="cos_sb")
        nc.sync.dma_start(cos_sb[:], cosd)
        sin_sb = const.tile([128, S], f16, tag="sin", name="sin_sb")
        nc.sync.dma_start(sin_sb[:], sind)
        nc.sync.dma_start(wq_sb[:, 2 * QW:4 * QW], wq[:, 2 * QW:4 * QW])
        nc.sync.dma_start(wk_sb[:, KW:], wk[:, KW:])
        nc.sync.dma_start(wv_sb[:, KW:], wv[:, KW:])
        nc.sync.dma_start(wq_sb[:, 4 * QW:], wq[:, 4 * QW:])
        fetch_half(1)

        # ---- wo resident (needed first at chunk 1) ---------------------
        wo_sb = const.tile([128, HQ * NE * CH], f16, tag="wo", name="wo_sb")
        nc.sync.dma_start(wo_sb[:], wo)

        # persistent rotated K (d-major) and V (t-major) for all tokens
        krotT = keep.tile([128, S], f16, tag="krotT", name="krotT")
        v_all = keep.tile([128, S], f16, tag="v_all", name="v_all")

        def rope(dst, src_ps, b_col, bs_col, s0):
            """dst = (src+b)*cos + swap(src+b)*sinS, straight from PSUM.
            sinS rows 0:64 = -sin, rows 64:128 = +sin; swap(b) passed in."""
            t1 = work.tile([128, CH], f16, tag="r1", bufs=2, name="t1")
            nc.vector.scalar_tensor_tensor(
                t1[:], src_ps[:], b_col, cos_sb[:, s0:s0 + CH],
                op0=AluOpType.add, op1=AluOpType.mult)
            sw = work.tile([128, CH], f16, tag="rsw", bufs=2, name="sw")
            nc.vector.tensor_copy(sw[0:64, :], src_ps[64:128, :])
            nc.vector.tensor_copy(sw[64:128, :], src_ps[0:64, :])
            t2 = work.tile([128, CH], f16, tag="r2", bufs=2, name="t2")
            nc.vector.scalar_tensor_tensor(
                t2[:], sw[:], bs_col, sin_sb[:, s0:s0 + CH],
                op0=AluOpType.add, op1=AluOpType.mult)
            nc.vector.tensor_tensor(dst, t1[:], t2[:], op=AluOpType.add)

        def attn_pass(pi, heads, qrot, blo, bhi, an_out):
            """Sliding-window attention for a head pair, j-outer,
            software-pipelined. at accumulators are copied to SBUF at the
            flush (frees the qkv PSUM banks); the normalization chain then
            overlaps the next chunk's qkv matmuls."""
            lg_tags = ("c0", "c1", "a3") if pi == 0 else ("a3", "b0", "b1")
            at = {}
            for idx, h in enumerate(heads):
                at[h] = ps.tile([128, CH], f32, tag=f"a{idx}", name="at_ps")
                nc.vector.memset(at[h][:], 0.0)
            dn = ps.tile([128, CH], f32, tag="a2", name="dn_ps")
            nc.vector.memset(dn[:], 0.0)
            lgi = 0
            pend = []

            def consume(item):
                idx, h, E, c0, w, j = item
                p = 64 * idx
                nc.tensor.matmul(dn[p:p + 1, c0:c0 + w], ones_c, E[:, :w],
                                 start=False, stop=True)
                nc.tensor.matmul(at[h][:, c0:c0 + w],
                                 v_all[:, j * 128:(j + 1) * 128],
                                 E[:, :w], start=False, stop=True)

            for j in range(max(0, blo - WBLK), bhi + 1):
                lo, hi = max(j, blo), min(j + WBLK, bhi)
                c0 = (lo - blo) * 128
                w = (hi - lo + 1) * 128
                has_m0 = lo == j
                has_m8 = hi == j + WBLK
                for idx, h in enumerate(heads):
                    lg = ps.tile([128, CH], f32, tag=lg_tags[lgi % len(lg_tags)],
                                 name="lg")
                    lgi += 1
                    nc.tensor.matmul(lg[:, :w],
                                     krotT[:, j * 128:(j + 1) * 128],
                                     qrot[h][:, c0:c0 + w],
                                     start=True, stop=True)
                    E = work.tile([128, CH], f16, tag="E", bufs=10, name="E")
                    nc.scalar.activation(E[:, :w], lg[:, :w], AF.Exp,
                                         scale=SCALE)
                    if has_m0:
                        nc.vector.tensor_tensor(E[:, 0:128], E[:, 0:128],
                                                m0_sb, op=AluOpType.mult)
                    if has_m8:
                        nc.vector.tensor_tensor(E[:, w - 128:w],
                                                E[:, w - 128:w],
                                                m8_sb, op=AluOpType.mult)
                    pend.append((idx, h, E, c0, w, j))
                while len(pend) > len(lg_tags) + 1:
                    consume(pend.pop(0))
            for item in pend:
                consume(item)

            # free the qkv PSUM banks immediately: at -> SBUF on scalar
            at_sb = {}
            for idx, h in enumerate(heads):
                t = work.tile([128, CH], f32, tag="atsb", bufs=4,
                              name="at_sb")
                nc.scalar.copy(t[:], at[h][:])
                at_sb[h] = t
            # normalization: rcp = 1/(dn + esink) batched for both heads;
            # overlaps the next chunk's qkv matmuls
            dnb = work.tile([128, CH], f32, tag="dnb", bufs=2, name="dnb")
            nc.vector.tensor_scalar_add(dnb[:], dn[:], es_sb[:, pi:pi + 1])
            rcp = work.tile([128, CH], f32, tag="rcp", bufs=2, name="rcp")
            nc.vector.reciprocal_approx_fast(rcp[:], dnb[:])
            rcp16 = work.tile([128, CH], f16, tag="rcp16", bufs=2,
                              name="rcp16")
            nc.vector.tensor_copy(rcp16[:], rcp[:])
            for idx, h in enumerate(heads):
                p = 64 * idx
                rb_ps = ps.tile([128, CH], f32, tag=("c0", "c1")[idx],
                                name="rb_ps")
                nc.tensor.matmul(rb_ps[:], ones_r[p:p + 1, :],
                                 rcp16[p:p + 1, :], start=True, stop=True)
                an = work.tile([128, CH], f16, tag="an", bufs=8, name="an")
                nc.vector.tensor_tensor(an[:], at_sb[h][:], rb_ps[:],
                                        op=AluOpType.mult)
                an_out[h] = an

        def o_proj(s0, an_out):
            for e in range(NE):
                for sb in range(CBLK):
                    o_ps = ps.tile([128, CH], f32,
                                   tag=("c0", "c1")[(e * CBLK + sb) % 2],
                                   name="o_ps")
                    for jt in range(HQ):
                        nc.tensor.matmul(
                            o_ps[:], an_out[jt][:, sb * 128:(sb + 1) * 128],
                            wo_sb[:, (jt * NE + e) * CH:(jt * NE + e + 1) * CH],
                            start=jt == 0, stop=jt == HQ - 1)
                    o_sb = work.tile([128, CH], f16, tag="osb", bufs=4,
                                     name="o_sb")
                    if (e + sb) % 2 == 0:
                        nc.scalar.copy(o_sb[:], o_ps[:])
                    else:
                        nc.vector.tensor_copy(o_sb[:], o_ps[:])
                    nc.sync.dma_start(
                        out[s0 + sb * 128:s0 + (sb + 1) * 128,
                            e * CH:(e + 1) * CH], o_sb[:])

        QTAG = ("a0", "a1", "b0", "b1")
        prev = None
        for c in range(NCH):
            s0 = c * CH

            # ---- fused q/k/v projection for this token chunk ----------
            q_ps = [ps.tile([128, CH], f32, tag=QTAG[jt], name=f"q_ps{jt}")
                    for jt in range(HQ)]
            k_ps = ps.tile([128, CH], f32, tag="a2", name="k_ps")
            v_ps = ps.tile([128, CH], f32, tag="a3", name="v_ps")
            for kt in range(KT):
                if kt == 0 and c + 1 < NCH:
                    fetch_half(2 * (c + 1))
                if kt == KH and c + 1 < NCH:
                    fetch_half(2 * (c + 1) + 1)
                hs_kt = hst_t[2 * c + kt // KH][
                    :, (kt % KH) * CH:(kt % KH + 1) * CH]
                first, last = kt == 0, kt == KT - 1
                for jt in range(HQ):
                    nc.tensor.matmul(
                        q_ps[jt][:],
                        wq_sb[:, kt * HQ * D + jt * D:kt * HQ * D + (jt + 1) * D],
                        hs_kt, start=first, stop=last)
                nc.tensor.matmul(k_ps[:], wk_sb[:, kt * D:(kt + 1) * D],
                                 hs_kt, start=first, stop=last)
                nc.tensor.matmul(v_ps[:], wv_sb[:, kt * D:(kt + 1) * D],
                                 hs_kt, start=first, stop=last)

            # ---- K + first head pair rope (vector) queued before o_proj
            rope(krotT[:, s0:s0 + CH], k_ps, bk_sb, bks_sb, s0)
            qrot = {}
            for jt in (0, 1):
                qr = work.tile([128, CH], f16, tag="qrot", bufs=4, name="qr")
                rope(qr[:], q_ps[jt], bq_sb[:, jt:jt + 1],
                     bqs_sb[:, jt:jt + 1], s0)
                qrot[jt] = qr
            # ---- V: bias (vector), PE transpose, scalar copy ----------
            vT_sb = work.tile([128, CH], f16, tag="vt", bufs=2, name="vT_sb")
            nc.vector.tensor_scalar_add(vT_sb[:], v_ps[:], bv_sb)
            for i in range(CBLK):
                vt = ps.tile([128, 128], f16, tag=("c0", "c1")[i % 2],
                             name="vt")
                nc.tensor.transpose(vt[:], vT_sb[:, i * 128:(i + 1) * 128],
                                    idf_sb)
                nc.scalar.copy(
                    v_all[:, s0 + i * 128:s0 + (i + 1) * 128], vt[:])

            # ---- o_proj of PREVIOUS chunk keeps the PE busy while the
            # ---- vector engine ropes ----------------------------------
            if prev is not None:
                o_proj(prev[0], prev[1])

            # ---- attention passes; second head pair ropes in between --
            blo, bhi = c * CBLK, c * CBLK + CBLK - 1
            an_out = {}
            attn_pass(0, (0, 1), qrot, blo, bhi, an_out)
            for jt in (2, 3):
                qr = work.tile([128, CH], f16, tag="qrot", bufs=4, name="qr")
                rope(qr[:], q_ps[jt], bq_sb[:, jt:jt + 1],
                     bqs_sb[:, jt:jt + 1], s0)
                qrot[jt] = qr
            attn_pass(1, (2, 3), qrot, blo, bhi, an_out)
            prev = (s0, an_out)
        o_proj(prev[0], prev[1])

    nc.compile()
    return nc


_CACHED = None
_LAST_IN_MAPS = None


def _get_nc():
    global _CACHED
    if _CACHED is None:
        _CACHED = _build()
    return _CACHED


def kernel(positions, hidden_states, Wq, bq, Wk, bk, Wv, bv, Wo, sink,
           **_ignored):
    positions = np.asarray(positions)
    hidden_states = np.asarray(hidden_states, dtype=np.float32)
    Wq = np.asarray(Wq, dtype=np.float32)
    Wk = np.asarray(Wk, dtype=np.float32)
    Wv = np.asarray(Wv, dtype=np.float32)
    Wo = np.asarray(Wo, dtype=np.float32)
    bq = np.asarray(bq, dtype=np.float32)
    bk = np.asarray(bk, dtype=np.float32)
    bv = np.asarray(bv, dtype=np.float32)
    sink = np.asarray(sink, dtype=np.float32)

    # host-derived tables
    half = D // 2
    inv_freq = 1.0 / (THETA ** (np.arange(half, dtype=np.float64) / half))
    ang = positions[0].astype(np.float64)[None, :] * inv_freq[:, None]  # [64,S]
    cos64 = np.cos(ang).astype(np.float16)
    sin64 = np.sin(ang)
    cosd = np.ascontiguousarray(np.concatenate([cos64, cos64], axis=0))
    # signed sin: top half -sin (x1c - x2s), bottom half +sin (x2c + x1s)
    sind = np.ascontiguousarray(
        np.concatenate([-sin64, sin64], axis=0)).astype(np.float16)
    r, cidx = np.arange(128)[:, None], np.arange(128)[None, :]
    m0 = (r <= cidx).astype(np.float16)
    m8 = (r > cidx).astype(np.float16)

    def swap_halves(b):  # [D] -> halves exchanged
        return np.concatenate([b[half:], b[:half]])

    # hsT half-chunk tiled: [NCH*2*128, KH*CH]
    hsT_full = np.ascontiguousarray(hidden_states[0].T).astype(np.float16)
    hsT_t = np.ascontiguousarray(
        hsT_full.reshape(2, KH, 128, NCH, CH).transpose(3, 0, 2, 1, 4)
        .reshape(NCH * 2 * 128, KH * CH))
    esink_all = np.exp(sink.astype(np.float64)).astype(np.float32)

    # packed f16 consts: [m0 m8 onc onr ident]
    cf16 = np.concatenate([
        m0, m8, np.ones((128, 1), dtype=np.float16),
        np.ones((128, 128), dtype=np.float16),
        np.eye(128, dtype=np.float16)], axis=1)
    cf16 = np.ascontiguousarray(cf16)

    in_maps = []
    for core in range(NCORES):
        qs = slice(core * HQ * D, (core + 1) * HQ * D)
        ks = slice(core * D, (core + 1) * D)
        # weights pre-tiled: [128, KT*cols] with kt blocks side by side
        wq_c = Wq[:, qs].astype(np.float16).reshape(KT, 128, HQ * D)
        wq_t = np.ascontiguousarray(
            wq_c.transpose(1, 0, 2).reshape(128, KT * HQ * D))
        wk_t = np.ascontiguousarray(
            Wk[:, ks].astype(np.float16).reshape(KT, 128, D)
            .transpose(1, 0, 2).reshape(128, KT * D))
        wv_t = np.ascontiguousarray(
            Wv[:, ks].astype(np.float16).reshape(KT, 128, D)
            .transpose(1, 0, 2).reshape(128, KT * D))
        # wo flat: [128, (jt*NE + e)*CH + x]
        wo_t = np.ascontiguousarray(
            Wo[qs, :].astype(np.float16).reshape(HQ, 128, NE * CH)
            .transpose(1, 0, 2).reshape(128, HQ * NE * CH))
        bq_c = bq[qs].reshape(HQ, D)
        bqs_c = np.stack([swap_halves(b) for b in bq_c])
        es = np.ones((128, 2), dtype=np.float32)
        for pi in range(2):
            es[0, pi] = esink_all[core * HQ + 2 * pi]
            es[64, pi] = esink_all[core * HQ + 2 * pi + 1]
        cf32 = np.concatenate([
            bq_c.T, bqs_c.T,
            bk[ks].reshape(D, 1), swap_halves(bk[ks]).reshape(D, 1),
            bv[ks].reshape(D, 1), es], axis=1).astype(np.float32)
        in_maps.append(dict(
            hsT=hsT_t, wq=wq_t, wk=wk_t, wv=wv_t, wo=wo_t,
            cf32=np.ascontiguousarray(cf32), cf16=cf16,
            cosd=cosd, sind=sind,
        ))

    global _LAST_IN_MAPS
    _LAST_IN_MAPS = in_maps
    nc = _get_nc()
    res = None
    for attempt in range(3):
        try:
            res = run_bass_kernel_spmd(nc, in_maps, list(range(NCORES)))
            break
        except Exception:
            if attempt == 2:
                raise
            import time as _t
            _t.sleep(2.0)
    out = np.zeros((S, HID), dtype=np.float64)
    for core in range(NCORES):
        out += res.results[core]["o_part"].astype(np.float64)
    return out.astype(np.float32).reshape(B, S, HID)
